# revision 8
# baseline (speedup 1.0000x reference)
"""Trainium2 Bass kernel for nn_AdvancedTransformerModel (dense transformer,
B=4, S=512, D=768, H=12, L=12, V=8000) on 8 NeuronCores.

Sharding: 2 groups x 4 cores; group g handles batches {2g, 2g+1}. Within a
group, core r owns query rows [128r, 128r+128) of each batch (token
sharding). Each core computes all 12 heads of the blended-attention path for
its own 128 query rows (the head average is local, so no attention
collective), K / phase-vector projections for the full sequence (duplicated,
cheap), and the full FFN for its own rows (no FFN reduction). One bf16
AllGather of the per-layer residual delta per layer per stream keeps the
residual stream replicated. All per-core differences are data-driven
(rotated ids/pos inputs and an index tensor for the gather-back), so a
single SPMD program runs on all 8 cores.

Layernorm gains/biases are folded into consuming matmul weights on the host.
Matmul operands are bf16; residual stream, PSUM accumulation and softmax
statistics stay fp32; the per-layer delta crosses the AllGather in bf16.
"""

from contextlib import ExitStack

import numpy as np

V, D, H, HD, FF, L, PP = 8000, 768, 12, 64, 3072, 12, 64
B, S = 4, 512
EPS = 1e-5
N_CORES = 8
CPG = 4           # cores per group
NSTREAM = 2       # batches per group
QT = S // 128     # 4 query tiles of 128 tokens
DT = D // 128     # 6 feature tiles
FFT = FF // 128   # 24 ffn feature tiles
NCH = 4           # ffn weight chunks
FTC = FFT // NCH  # 8 ffn tiles per chunk
VPC = V // CPG    # vocab cols per core = 2000
REPLICA_GROUPS = [[0, 1, 2, 3], [4, 5, 6, 7]]

_PROGRAM_CACHE = {}


# ----------------------------------------------------------------- program
def _build_program(add_b2: bool, g1_nontrivial: bool, add_bq: bool,
                   n_layers: int = L, no_cc: bool = False, dbg: bool = False):
    import concourse.bass as bass
    import concourse.mybir as mybir
    from concourse import bacc
    from concourse.tile import TileContext
    from concourse.masks import make_identity

    dt = mybir.dt
    f32, bf, i32 = dt.float32, dt.bfloat16, dt.int32
    Alu = mybir.AluOpType
    Act = mybir.ActivationFunctionType

    nc = bacc.Bacc("TRN2", target_bir_lowering=False, debug=False,
                   num_devices=N_CORES)

    # ---- I/O -------------------------------------------------------------
    IDS = nc.dram_tensor("ids", [NSTREAM, S], i32, kind="ExternalInput")
    YIDX = nc.dram_tensor("yidx", [128, NSTREAM * QT], i32, kind="ExternalInput")
    TOK = nc.dram_tensor("tok", [V, D], bf, kind="ExternalInput")
    POS = nc.dram_tensor("pos", [128, QT * D], f32, kind="ExternalInput")
    WQK = nc.dram_tensor("wqk", [L, 128, DT * 2 * D], bf, kind="ExternalInput")
    BK = nc.dram_tensor("bk", [L, DT, 128], f32, kind="ExternalInput")
    WP = nc.dram_tensor("wp", [L, 128, DT * PP], bf, kind="ExternalInput")
    BPV = nc.dram_tensor("bpv", [L, PP], f32, kind="ExternalInput")
    W1 = nc.dram_tensor("w1", [L, 128, DT * FF], bf, kind="ExternalInput")
    B1 = nc.dram_tensor("b1", [L, FFT, 128], f32, kind="ExternalInput")
    W2 = nc.dram_tensor("w2", [L, 128, FFT * D], bf, kind="ExternalInput")
    AH = nc.dram_tensor("ah", [L], f32, kind="ExternalInput")
    WO = nc.dram_tensor("wo", [128, DT * VPC], bf, kind="ExternalInput")
    TICK = nc.dram_tensor("tick", [1, 4], f32, kind="ExternalInput")
    BQ = (nc.dram_tensor("bq", [L, D], f32, kind="ExternalInput")
          if add_bq else None)
    B2Q = (nc.dram_tensor("b2q", [L, D], f32, kind="ExternalInput")
           if add_b2 else None)
    G1B1 = (nc.dram_tensor("g1b1", [L, 2, D], f32, kind="ExternalInput")
            if g1_nontrivial else None)

    OUT = nc.dram_tensor("out", [NSTREAM, S, VPC], f32, kind="ExternalOutput")
    TICKO = nc.dram_tensor("ticko", [1, 4], f32, kind="ExternalOutput")
    DBG = (nc.dram_tensor("dbg", [NSTREAM, 6, 128, QT, D], f32,
                          kind="ExternalOutput") if dbg else None)

    with TileContext(nc) as tc, ExitStack() as ctx:
        dram_cc = ctx.enter_context(tc.tile_pool(name="ccd", bufs=1,
                                                 space="DRAM"))
        ccagi = [dram_cc.tile([NSTREAM * 128, D], bf, name=f"ccagi_{l}")
                 for l in range(n_layers)]
        ccago = [dram_cc.tile([CPG * NSTREAM * 128, D], bf,
                              name=f"ccago_{l}")
                 for l in range(n_layers)]
        persist = ctx.enter_context(tc.tile_pool(name="persist", bufs=1))
        wts = ctx.enter_context(tc.tile_pool(name="wts", bufs=2))
        wf = ctx.enter_context(tc.tile_pool(name="wf", bufs=2))
        h1p = ctx.enter_context(tc.tile_pool(name="h1p", bufs=2))
        tmp = ctx.enter_context(tc.tile_pool(name="tmp", bufs=2))
        eh_pool = ctx.enter_context(tc.tile_pool(name="eh", bufs=3))
        yf_pool = ctx.enter_context(tc.tile_pool(name="yf", bufs=1))
        wo_pool = ctx.enter_context(tc.tile_pool(name="wo", bufs=1))
        ps_s = ctx.enter_context(tc.tile_pool(name="ps_s", bufs=2, space="PSUM"))
        ps_t = ctx.enter_context(tc.tile_pool(name="ps_t", bufs=2, space="PSUM"))
        ps_f = ctx.enter_context(tc.tile_pool(name="ps_f", bufs=2, space="PSUM"))
        ps_w = ctx.enter_context(tc.tile_pool(name="ps_w", bufs=2, space="PSUM"))

        # --- constants / persistent state --------------------------------
        ident = persist.tile([128, 128], bf)
        make_identity(nc, ident[:])
        epst = persist.tile([128, 1], f32)
        nc.vector.memset(epst[:], EPS)

        x_t = [persist.tile([128, QT, D], f32, name=f"x_{s}")
               for s in range(NSTREAM)]
        xn_t = [persist.tile([128, QT, D], bf, name=f"xn_{s}")
                for s in range(NSTREAM)]
        xnT_t = [persist.tile([128, DT, S], bf, name=f"xnT_{s}")
                 for s in range(NSTREAM)]
        kf_t = [persist.tile([128, DT, S], bf, name=f"kf_{s}")
                for s in range(NSTREAM)]
        pvT_t = [persist.tile([64, S], bf, name=f"pvT_{s}")
                 for s in range(NSTREAM)]
        qT_t = persist.tile([128, DT, NSTREAM * 128], bf, name="qT")
        xn2T_t = persist.tile([128, DT, NSTREAM * 128], bf, name="xn2T")
        aw_t = [persist.tile([128, S], f32, name=f"aw_{s}")
                for s in range(NSTREAM)]
        epc_t = [persist.tile([128, S], bf, name=f"epc_{s}")
                 for s in range(NSTREAM)]
        num_t = [persist.tile([128, S], bf, name=f"num_{s}")
                 for s in range(NSTREAM)]
        numT_t = [persist.tile([128, QT, 128], bf, name=f"numT_{s}")
                  for s in range(NSTREAM)]
        delta_t = [persist.tile([128, D], f32, name=f"delta_{s}")
                   for s in range(NSTREAM)]
        xmid_t = [persist.tile([128, D], f32, name=f"xmid_{s}")
                  for s in range(NSTREAM)]
        Zs_t = [persist.tile([128, H], f32, name=f"Zs_{s}")
                for s in range(NSTREAM)]
        rZ_t = [persist.tile([128, H], f32, name=f"rZ_{s}")
                for s in range(NSTREAM)]
        Zb_t = [persist.tile([128, 2], f32, name=f"Zb_{s}")
                for s in range(NSTREAM)]
        bnst_t = [persist.tile([128, QT, 3, 6], f32, name=f"bnst_{s}")
                  for s in range(NSTREAM)]
        mv_t = [persist.tile([128, QT, 2], f32, name=f"mv_{s}")
                for s in range(NSTREAM)]
        sc_t = [persist.tile([128, 2, QT], f32, name=f"sc_{s}")
                for s in range(NSTREAM)]

        # tick passthrough (timing harness dependency hook)
        tick_s = persist.tile([1, 4], f32)
        nc.sync.dma_start(out=tick_s[:], in_=TICK.ap())
        nc.sync.dma_start(out=TICKO.ap(), in_=tick_s[:])

        # --- embedding ----------------------------------------------------
        ids_s = persist.tile([128, NSTREAM, QT], i32)
        nc.sync.dma_start(
            out=ids_s[:], in_=IDS.ap().rearrange("s (t p) -> p s t", p=128))
        yidx_s = persist.tile([128, NSTREAM * QT], i32)
        nc.sync.dma_start(out=yidx_s[:], in_=YIDX.ap())
        for s in range(NSTREAM):
            for qt in range(QT):
                tok_tile = tmp.tile([128, D], bf, tag="tok")
                nc.gpsimd.indirect_dma_start(
                    out=tok_tile[:], out_offset=None, in_=TOK.ap(),
                    in_offset=bass.IndirectOffsetOnAxis(
                        ap=ids_s[:, s, qt:qt + 1], axis=0),
                )
                pos_tile = tmp.tile([128, D], f32, tag="pos")
                nc.sync.dma_start(
                    out=pos_tile[:],
                    in_=POS.ap().rearrange("p (t f) -> p t f", t=QT)[:, qt, :])
                nc.vector.tensor_tensor(
                    out=x_t[s][:, qt, :], in0=tok_tile[:], in1=pos_tile[:],
                    op=Alu.add)
        if dbg:
            for s in range(NSTREAM):
                nc.sync.dma_start(out=DBG.ap()[s][0], in_=x_t[s][:])

        # --- helpers ------------------------------------------------------
        def layernorm(s, src, out_bf, nqt):
            """src [128, nqt, D] f32 -> out_bf [128, nqt, D] bf16 normalized
            per row. One-pass stats on DVE via bn_stats (3 subgroups of 256).
            """
            bnst, mv, sc = bnst_t[s], mv_t[s], sc_t[s]
            for qt in range(nqt):
                r3 = src[:, qt, :].rearrange("p (a b) -> p a b", a=3)
                for sub in range(3):
                    nc.vector.bn_stats(out=bnst[:, qt, sub, :],
                                       in_=r3[:, sub, :])
                nc.vector.bn_aggr(out=mv[:, qt, :], in_=bnst[:, qt, :, :])
                # rs = rsqrt(var + eps); nm = mean * rs
                nc.scalar.activation(out=sc[:, 0, qt:qt + 1],
                                     in_=mv[:, qt, 1:2],
                                     func=Act.Sqrt, bias=epst[:])
                nc.vector.reciprocal(out=sc[:, 0, qt:qt + 1],
                                     in_=sc[:, 0, qt:qt + 1])
                nc.vector.tensor_tensor(out=sc[:, 1, qt:qt + 1],
                                        in0=mv[:, qt, 0:1],
                                        in1=sc[:, 0, qt:qt + 1], op=Alu.mult)
                nc.vector.tensor_scalar(
                    out=out_bf[:, qt, :], in0=src[:, qt, :],
                    scalar1=sc[:, 0, qt:qt + 1], scalar2=sc[:, 1, qt:qt + 1],
                    op0=Alu.mult, op1=Alu.subtract)

        def layernorm1(s, src, out_bf):
            """src [128, D] f32 -> out_bf [128, D] bf16 (single tile)."""
            bnst, mv, sc = bnst_t[s], mv_t[s], sc_t[s]
            r3 = src.rearrange("p (a b) -> p a b", a=3)
            for sub in range(3):
                nc.vector.bn_stats(out=bnst[:, 0, sub, :], in_=r3[:, sub, :])
            nc.vector.bn_aggr(out=mv[:, 0, :], in_=bnst[:, 0, :, :])
            nc.scalar.activation(out=sc[:, 0, 0:1], in_=mv[:, 0, 1:2],
                                 func=Act.Sqrt, bias=epst[:])
            nc.vector.reciprocal(out=sc[:, 0, 0:1], in_=sc[:, 0, 0:1])
            nc.vector.tensor_tensor(out=sc[:, 1, 0:1], in0=mv[:, 0, 0:1],
                                    in1=sc[:, 0, 0:1], op=Alu.mult)
            nc.vector.tensor_scalar(
                out=out_bf, in0=src, scalar1=sc[:, 0, 0:1],
                scalar2=sc[:, 1, 0:1], op0=Alu.mult, op1=Alu.subtract)

        def transpose_qd(src, dst, nt):
            """src [128, QT, nt*128] bf16 -> dst [128, nt, S] bf16."""
            for dtile in range(nt):
                tp = ps_t.tile([128, S], bf, tag="tp")
                for qt in range(QT):
                    nc.tensor.transpose(
                        tp[:, qt * 128:(qt + 1) * 128],
                        src[:, qt, dtile * 128:(dtile + 1) * 128],
                        ident[:])
                nc.vector.tensor_copy(dst[:, dtile, :], tp[:])

        # --- attention (per stream) --------------------------------------
        def stage_attn(l, s, wqk_l, bk_l, bq_l, wp_l, bpv_l, ah_l, g1b1_l):
            layernorm(s, x_t[s][:], xn_t[s][:], QT)
            transpose_qd(xn_t[s][:], xnT_t[s][:], DT)
            # K projection, full sequence, feature-major
            for ft in range(DT):
                ps = ps_s.tile([128, S], f32, tag="ps")
                for dtile in range(DT):
                    nc.tensor.matmul(
                        ps[:], wqk_l[:, dtile, ft * 128:(ft + 1) * 128],
                        xnT_t[s][:, dtile, :],
                        start=(dtile == 0), stop=(dtile == DT - 1))
                nc.scalar.activation(out=kf_t[s][:, ft, :], in_=ps[:],
                                     func=Act.Identity,
                                     bias=bk_l[:, ft:ft + 1])
            # phase-vector projection, full sequence
            ps = ps_s.tile([128, S], f32, tag="ps")
            for dtile in range(DT):
                nc.tensor.matmul(ps[:PP, :], wp_l[:, dtile, :],
                                 xnT_t[s][:, dtile, :],
                                 start=(dtile == 0), stop=(dtile == DT - 1))
            nc.scalar.activation(out=pvT_t[s][:, :], in_=ps[:PP, :],
                                 func=Act.Tanh, bias=bpv_l[:, 0:1])
            # Q for own 128 rows: token-major, then transpose to feature-major
            qrow = tmp.tile([128, D], bf, tag="qrow")
            for half in range(2):
                ps = ps_w.tile([128, 384], f32, tag="psw")
                for dtile in range(DT):
                    nc.tensor.matmul(
                        ps[:], xnT_t[s][:, dtile, 0:128],
                        wqk_l[:, dtile, D + half * 384:D + (half + 1) * 384],
                        start=(dtile == 0), stop=(dtile == DT - 1))
                if bq_l is not None:
                    nc.vector.tensor_tensor(
                        out=qrow[:, half * 384:(half + 1) * 384], in0=ps[:],
                        in1=bq_l[:, half * 384:(half + 1) * 384], op=Alu.add)
                else:
                    nc.vector.tensor_copy(
                        qrow[:, half * 384:(half + 1) * 384], ps[:])
            tpA = ps_t.tile([128, S], bf, tag="tp")
            for dtile in range(4):
                nc.tensor.transpose(
                    tpA[:, dtile * 128:(dtile + 1) * 128],
                    qrow[:, dtile * 128:(dtile + 1) * 128], ident[:])
            nc.vector.tensor_copy(
                qT_t[:, 0:4, s * 128:(s + 1) * 128],
                tpA[:].rearrange("p (t f) -> p t f", t=4))
            tpB = ps_t.tile([128, S], bf, tag="tp")
            for dtile in range(2):
                nc.tensor.transpose(
                    tpB[:, dtile * 128:(dtile + 1) * 128],
                    qrow[:, (4 + dtile) * 128:(5 + dtile) * 128], ident[:])
            nc.vector.tensor_copy(
                qT_t[:, 4:6, s * 128:(s + 1) * 128],
                tpB[:, 0:256].rearrange("p (t f) -> p t f", t=2))
            # scores: all 12 heads for own 128 query rows
            Zs, rZ, aw = Zs_t[s], rZ_t[s], aw_t[s]
            for h in range(H):
                po = (h % 2) * 64
                ftile = h // 2
                ps = ps_s.tile([128, S], f32, tag="ps")
                nc.tensor.matmul(
                    ps[:], qT_t[po:po + 64, ftile, s * 128:(s + 1) * 128],
                    kf_t[s][po:po + 64, ftile, :], start=True, stop=True,
                    tile_position=(po, 0))
                eh = eh_pool.tile([128, S], bf, tag="eh")
                nc.scalar.activation(
                    out=eh[:], in_=ps[:], func=Act.Exp,
                    scale=float(1.0 / np.sqrt(HD)),
                    accum_out=Zs[:, h:h + 1])
                nc.vector.tensor_scalar(
                    out=rZ[:, h:h + 1], in0=Zs[:, h:h + 1],
                    scalar1=float(H), scalar2=None, op0=Alu.mult)
                nc.vector.reciprocal(out=rZ[:, h:h + 1], in_=rZ[:, h:h + 1])
                if h == 0:
                    nc.vector.tensor_scalar(
                        out=aw[:], in0=eh[:], scalar1=rZ[:, h:h + 1],
                        scalar2=None, op0=Alu.mult)
                else:
                    nc.vector.scalar_tensor_tensor(
                        out=aw[:], in0=eh[:], scalar=rZ[:, h:h + 1],
                        in1=aw[:], op0=Alu.mult, op1=Alu.add)
            # blend with phase coherence for own rows
            ps = ps_s.tile([128, S], f32, tag="ps")
            nc.tensor.matmul(ps[:], pvT_t[s][:, 0:128], pvT_t[s][:, :],
                             start=True, stop=True)
            nc.scalar.activation(out=epc_t[s][:], in_=ps[:], func=Act.Exp,
                                 scale=ah_l[:, 0:1], bias=ah_l[:, 0:1])
            nc.vector.scalar_tensor_tensor(
                out=num_t[s][:], in0=aw[:], scalar=1e-6, in1=epc_t[s][:],
                op0=Alu.add, op1=Alu.mult, accum_out=Zb_t[s][:, 0:1])
            nc.vector.reciprocal(out=Zb_t[s][:, 1:2], in_=Zb_t[s][:, 0:1])
            # transpose num -> numT [128k, kt, 128q]
            tp = ps_t.tile([128, S], bf, tag="tp")
            for kt in range(QT):
                nc.tensor.transpose(
                    tp[:, kt * 128:(kt + 1) * 128],
                    num_t[s][:, kt * 128:(kt + 1) * 128], ident[:])
            nc.vector.tensor_copy(
                numT_t[s][:],
                tp[:].rearrange("p (t f) -> p t f", t=4))
            if dbg and l == 0:
                awf = yf_pool.tile([128, QT, 128], f32, tag="awdbg")
                nc.vector.tensor_copy(
                    awf[:], num_t[s][:].rearrange("p (t f) -> p t f", t=4))
                nc.sync.dma_start(out=DBG.ap()[s][3][:, :, :128], in_=awf[:])
            # attention output for own rows: delta = (numT.T @ vsrc) * rzb
            vsrc = xn_t[s]
            if g1b1_l is not None:
                vsrc = persist.tile([128, QT, D], bf, name=f"xnv_{s}")
                for qt in range(QT):
                    nc.vector.tensor_tensor(
                        out=vsrc[:, qt, :], in0=xn_t[s][:, qt, :],
                        in1=g1b1_l[:, 0, :], op=Alu.mult)
                    nc.vector.tensor_tensor(
                        out=vsrc[:, qt, :], in0=vsrc[:, qt, :],
                        in1=g1b1_l[:, 1, :], op=Alu.add)
            for half in range(2):
                ps = ps_w.tile([128, 384], f32, tag="psw")
                for kt in range(QT):
                    nc.tensor.matmul(
                        ps[:], numT_t[s][:, kt, :],
                        vsrc[:, kt, half * 384:(half + 1) * 384],
                        start=(kt == 0), stop=(kt == QT - 1))
                nc.vector.tensor_scalar(
                    out=delta_t[s][:, half * 384:(half + 1) * 384],
                    in0=ps[:], scalar1=Zb_t[s][:, 1:2], scalar2=None,
                    op0=Alu.mult)
            # xmid = x_own + attn delta; LN2; transpose for the FFN
            nc.gpsimd.tensor_tensor(
                out=xmid_t[s][:], in0=x_t[s][:, 0, :], in1=delta_t[s][:],
                op=Alu.add)
            if dbg and l == 0:
                nc.sync.dma_start(out=DBG.ap()[s][1][:, 0, :],
                                  in_=xmid_t[s][:])
            layernorm1(s, xmid_t[s][:], xn_t[s][:, 0, :])
            tpA = ps_t.tile([128, S], bf, tag="tp")
            for dtile in range(4):
                nc.tensor.transpose(
                    tpA[:, dtile * 128:(dtile + 1) * 128],
                    xn_t[s][:, 0, dtile * 128:(dtile + 1) * 128], ident[:])
            nc.vector.tensor_copy(
                xn2T_t[:, 0:4, s * 128:(s + 1) * 128],
                tpA[:].rearrange("p (t f) -> p t f", t=4))
            tpB = ps_t.tile([128, S], bf, tag="tp")
            for dtile in range(2):
                nc.tensor.transpose(
                    tpB[:, dtile * 128:(dtile + 1) * 128],
                    xn_t[s][:, 0, (4 + dtile) * 128:(5 + dtile) * 128],
                    ident[:])
            nc.vector.tensor_copy(
                xn2T_t[:, 4:6, s * 128:(s + 1) * 128],
                tpB[:, 0:256].rearrange("p (t f) -> p t f", t=2))

        # --- token-split FFN (both streams fused), then AllGather ---------
        def stage_ffn(l, b1f_l, b2_l):
            for c in range(NCH):
                w1c = wf.tile([128, DT, FF // NCH], bf, tag="w1c")
                nc.sync.dma_start(
                    out=w1c[:],
                    in_=W1.ap()[l].rearrange("p (t f) -> p t f", t=DT)
                    [:, :, c * (FF // NCH):(c + 1) * (FF // NCH)])
                w2c = wf.tile([128, FTC, D], bf, tag="w2c")
                nc.sync.dma_start(
                    out=w2c[:],
                    in_=W2.ap()[l].rearrange("p (t f) -> p t f", t=FFT)
                    [:, c * FTC:(c + 1) * FTC, :])
                h1c = h1p.tile([128, FTC, NSTREAM * 128], bf, tag="h1c")
                for ft in range(FTC):
                    ps = ps_f.tile([128, NSTREAM * 128], f32, tag="psf")
                    for dtile in range(DT):
                        nc.tensor.matmul(
                            ps[:], w1c[:, dtile, ft * 128:(ft + 1) * 128],
                            xn2T_t[:, dtile, :],
                            start=(dtile == 0), stop=(dtile == DT - 1))
                    nc.scalar.activation(
                        out=h1c[:, ft, :], in_=ps[:], func=Act.Gelu,
                        bias=b1f_l[:, c * FTC + ft:c * FTC + ft + 1])
                for s in range(NSTREAM):
                    for half in range(2):
                        ps = ps_w.tile([128, 384], f32, tag="psw")
                        for ft in range(FTC):
                            nc.tensor.matmul(
                                ps[:], h1c[:, ft, s * 128:(s + 1) * 128],
                                w2c[:, ft, half * 384:(half + 1) * 384],
                                start=(ft == 0), stop=(ft == FTC - 1))
                        nc.vector.tensor_tensor(
                            out=delta_t[s][:, half * 384:(half + 1) * 384],
                            in0=delta_t[s][:, half * 384:(half + 1) * 384],
                            in1=ps[:], op=Alu.add)
            for s in range(NSTREAM):
                if b2_l is not None:
                    nc.gpsimd.tensor_tensor(
                        out=delta_t[s][:], in0=delta_t[s][:], in1=b2_l[:],
                        op=Alu.add)
                yst = tmp.tile([128, D], bf, tag="yst")
                nc.gpsimd.tensor_copy(yst[:], delta_t[s][:])
                nc.sync.dma_start(out=ccagi[l][s * 128:(s + 1) * 128, :],
                                  in_=yst[:])
            if not no_cc:
                nc.gpsimd.collective_compute(
                    "AllGather", mybir.AluOpType.bypass,
                    replica_groups=REPLICA_GROUPS,
                    ins=[ccagi[l][:]], outs=[ccago[l][:]])

        def stage_update(l, s):
            yin = yf_pool.tile([128, QT, D], bf, tag="yin")
            for j in range(QT):
                if no_cc:
                    nc.sync.dma_start(
                        out=yin[:, j, :],
                        in_=ccagi[l][s * 128:(s + 1) * 128, :])
                else:
                    nc.gpsimd.indirect_dma_start(
                        out=yin[:, j, :], out_offset=None, in_=ccago[l][:],
                        in_offset=bass.IndirectOffsetOnAxis(
                            ap=yidx_s[:, s * QT + j:s * QT + j + 1],
                            axis=0))
            nc.gpsimd.tensor_tensor(
                out=x_t[s][:], in0=x_t[s][:], in1=yin[:], op=Alu.add)
            if dbg and l == 0:
                nc.sync.dma_start(out=DBG.ap()[s][2], in_=x_t[s][:])

        # --- layer loop ---------------------------------------------------
        for l in range(n_layers):
            wqk_l = wts.tile([128, DT, 2 * D], bf, tag="wqk")
            nc.sync.dma_start(
                out=wqk_l[:],
                in_=WQK.ap()[l].rearrange("p (t f) -> p t f", t=DT))
            bk_l = wts.tile([128, DT], f32, tag="bk")
            nc.sync.dma_start(
                out=bk_l[:], in_=BK.ap()[l].rearrange("t p -> p t"))
            wp_l = wts.tile([128, DT, PP], bf, tag="wp")
            nc.sync.dma_start(
                out=wp_l[:],
                in_=WP.ap()[l].rearrange("p (t f) -> p t f", t=DT))
            bpv_l = wts.tile([64, 1], f32, tag="bpv")
            nc.sync.dma_start(
                out=bpv_l[:],
                in_=BPV.ap()[l].rearrange("(t p) -> p t", p=64))
            b1f_l = wts.tile([128, FFT], f32, tag="b1f")
            nc.sync.dma_start(
                out=b1f_l[:], in_=B1.ap()[l].rearrange("t p -> p t"))
            ah_l = wts.tile([128, 1], f32, tag="ah")
            nc.sync.dma_start(
                out=ah_l[:],
                in_=bass.AP(tensor=AH.ap().tensor, offset=l,
                            ap=[[0, 128], [1, 1]]))
            bq_l = None
            if add_bq:
                bq_l = wts.tile([128, D], f32, tag="bq")
                nc.sync.dma_start(
                    out=bq_l[:],
                    in_=bass.AP(tensor=BQ.ap().tensor, offset=l * D,
                                ap=[[0, 128], [1, D]]))
            b2_l = None
            if add_b2:
                b2_l = wts.tile([128, D], f32, tag="b2")
                nc.sync.dma_start(
                    out=b2_l[:],
                    in_=bass.AP(tensor=B2Q.ap().tensor, offset=l * D,
                                ap=[[0, 128], [1, D]]))
            g1b1_l = None
            if g1_nontrivial:
                g1b1_l = wts.tile([128, 2, D], f32, tag="g1b1")
                nc.sync.dma_start(
                    out=g1b1_l[:],
                    in_=bass.AP(tensor=G1B1.ap().tensor, offset=l * 2 * D,
                                ap=[[0, 128], [D, 2], [1, D]]))

            for s in range(NSTREAM):
                stage_attn(l, s, wqk_l, bk_l, bq_l, wp_l, bpv_l, ah_l,
                           g1b1_l)
            stage_ffn(l, b1f_l, b2_l)
            for s in range(NSTREAM):
                stage_update(l, s)

        # --- final layernorm + logits ------------------------------------
        NSL = 4           # vocab column slices of 500
        for s in range(NSTREAM):
            layernorm(s, x_t[s][:], xn_t[s][:], QT)
            transpose_qd(xn_t[s][:], xnT_t[s][:], DT)
        for nsl in range(NSL):
            wo_sl = wo_pool.tile([128, DT, VPC // NSL], bf, tag="wo")
            nc.sync.dma_start(
                out=wo_sl[:],
                in_=WO.ap().rearrange("p (t f) -> p t f", t=DT)
                [:, :, nsl * 500:(nsl + 1) * 500])
            for s in range(NSTREAM):
                for qt in range(QT):
                    ps = ps_s.tile([128, S], f32, tag="ps")
                    for dtile in range(DT):
                        nc.tensor.matmul(
                            ps[:, :500],
                            xnT_t[s][:, dtile, qt * 128:(qt + 1) * 128],
                            wo_sl[:, dtile, :],
                            start=(dtile == 0), stop=(dtile == DT - 1))
                    lgst = tmp.tile([128, 500], f32, tag="lgst")
                    nc.scalar.activation(out=lgst[:], in_=ps[:, :500],
                                         func=Act.Copy)
                    nc.scalar.dma_start(
                        out=OUT.ap()[s].rearrange(
                            "(t p) v -> p t v",
                            p=128)[:, qt, nsl * 500:(nsl + 1) * 500],
                        in_=lgst[:])

    nc.compile()
    return nc


# ------------------------------------------------------------------ runner
class _Runner:
    def __init__(self, nc):
        import jax
        import concourse.mybir as mybir
        from jax.sharding import Mesh, PartitionSpec, NamedSharding
        from jax.experimental.shard_map import shard_map
        from concourse.bass2jax import (
            _bass_exec_p, install_neuronx_cc_hook, partition_id_tensor,
            fast_dispatch_compile)

        install_neuronx_cc_hook()
        self.nc = nc
        partition_name = (nc.partition_id_tensor.name
                          if nc.partition_id_tensor else None)
        in_names, out_names, out_avals, zero_outs = [], [], [], []
        in_shapes = {}
        for alloc in nc.m.functions[0].allocations:
            if not isinstance(alloc, mybir.MemoryLocationSet):
                continue
            name = alloc.memorylocations[0].name
            if alloc.kind == "ExternalInput":
                if name != partition_name:
                    in_names.append(name)
                    in_shapes[name] = (tuple(alloc.tensor_shape),
                                      mybir.dt.np(alloc.dtype))
            elif alloc.kind == "ExternalOutput":
                out_names.append(name)
                shape = tuple(alloc.tensor_shape)
                npdt = mybir.dt.np(alloc.dtype)
                out_avals.append(jax.core.ShapedArray(shape, npdt))
                zero_outs.append(np.zeros(shape, npdt))
        self.param_names, self.out_names = list(in_names), out_names
        n_params, n_outs = len(in_names), len(out_avals)
        all_in = list(in_names) + list(out_names)
        if partition_name is not None:
            all_in.append(partition_name)

        def _body(*args):
            operands = list(args)
            if partition_name is not None:
                operands.append(partition_id_tensor())
            return tuple(_bass_exec_p.bind(
                *operands, out_avals=tuple(out_avals),
                in_names=tuple(all_in), out_names=tuple(out_names),
                lowering_input_output_aliases=(),
                sim_require_finite=True, sim_require_nnan=True, nc=nc))

        devices = jax.devices("axon")[:N_CORES]
        self.mesh = Mesh(np.asarray(devices), ("core",))
        sharding = NamedSharding(self.mesh, PartitionSpec("core"))
        global_avals = []
        for name in in_names:
            shape, npdt = in_shapes[name]
            gshape = (N_CORES * shape[0],) + shape[1:]
            global_avals.append(
                jax.ShapeDtypeStruct(gshape, npdt, sharding=sharding))
        for z in zero_outs:
            gshape = (N_CORES * z.shape[0],) + z.shape[1:]
            global_avals.append(
                jax.ShapeDtypeStruct(gshape, z.dtype, sharding=sharding))

        def compile_fn():
            jitted = jax.jit(
                shard_map(
                    _body, mesh=self.mesh,
                    in_specs=(PartitionSpec("core"),) * (n_params + n_outs),
                    out_specs=(PartitionSpec("core"),) * n_outs,
                    check_rep=False),
                keep_unused=True)
            return jitted.lower(*global_avals).compile()

        self.fn = fast_dispatch_compile(compile_fn)
        self.zero_outs = zero_outs
        self._jax = jax
        self._P = PartitionSpec

    def put_inputs(self, in_maps):
        jax = self._jax
        sharding = jax.sharding.NamedSharding(self.mesh, self._P("core"))
        args = []
        for name in self.param_names:
            cat = np.concatenate(
                [np.asarray(in_maps[c][name]) for c in range(N_CORES)],
                axis=0)
            args.append(jax.device_put(cat, sharding))
        for z in self.zero_outs:
            cat = np.zeros((N_CORES * z.shape[0], *z.shape[1:]), z.dtype)
            args.append(jax.device_put(cat, sharding))
        return args

    def run(self, args):
        jax = self._jax
        outs = self.fn(*args)
        jax.block_until_ready(outs)
        res = []
        for c in range(N_CORES):
            d = {}
            for i, name in enumerate(self.out_names):
                full = np.asarray(outs[i])
                per = full.reshape(N_CORES, full.shape[0] // N_CORES,
                                   *full.shape[1:])
                d[name] = per[c]
            res.append(d)
        return res


def _get_runner(key, **build_kwargs):
    if key not in _PROGRAM_CACHE:
        nc = _build_program(*key, **build_kwargs)
        _PROGRAM_CACHE[key] = _Runner(nc)
    return _PROGRAM_CACHE[key]


# ----------------------------------------------------------- host wrapper
def _pmajor(a):
    """[D=nt*128, F] -> [128, nt*F] (per-partition contiguous)."""
    d, f = a.shape
    nt = d // 128
    return np.ascontiguousarray(
        a.reshape(nt, 128, f).transpose(1, 0, 2).reshape(128, nt * f))


def _prep_inputs(input_ids, tok_embed, pos_embed, ln1_g, ln1_b, w_in, b_in,
                 w_phase, alpha, ln2_g, ln2_b, w_ff1, b_ff1, w_ff2, b_ff2,
                 lnf_g, lnf_b, w_out, b_out):
    import ml_dtypes
    bft = ml_dtypes.bfloat16
    f32 = np.float32

    tok_bf = np.ascontiguousarray(np.asarray(tok_embed).astype(bft))
    pos_full = np.ascontiguousarray(np.asarray(pos_embed)[0, :S].astype(f32))
    alpha_h = (np.asarray(alpha).astype(f32) / 2.0)

    add_b2 = bool(np.any(b_ff2 != 0))
    g1_triv = (np.all(ln1_g == 1.0) and np.all(ln1_b == 0.0))

    # per-layer folds (fp32 host math)
    wqk_all, bq_all, bk_all, wp_all, bpv_all = [], [], [], [], []
    w1_all, b1_all, w2_all, b2_all, g1b1_all = [], [], [], [], []
    for l in range(L):
        g1, bb1 = ln1_g[l].astype(f32), ln1_b[l].astype(f32)
        g2, bb2 = ln2_g[l].astype(f32), ln2_b[l].astype(f32)
        wi = w_in[l].astype(f32)                    # [3D, D]
        wq_rows, wk_rows = wi[:D], wi[D:2 * D]
        bq = bb1 @ wq_rows.T + b_in[l][:D].astype(f32)
        bk = bb1 @ wk_rows.T + b_in[l][D:2 * D].astype(f32)
        wq_g = wq_rows * g1[None, :]
        wk_g = wk_rows * g1[None, :]
        # columns 0..D = K features; D..2D = Q features
        wqk_all.append(np.concatenate([wk_g.T, wq_g.T], axis=1))  # [D, 2D]
        bq_all.append(bq)
        bk_all.append(bk)
        wp_l = w_phase[l].astype(f32)               # [P, D]
        bpv_all.append(bb1 @ wp_l.T)
        wp_all.append(wp_l * g1[None, :])
        w1_l = w_ff1[l].astype(f32)                 # [FF, D]
        b1_all.append(bb2 @ w1_l.T + b_ff1[l].astype(f32))
        w1_all.append((w1_l * g2[None, :]).T)       # [D, FF]
        w2_all.append(w_ff2[l].astype(f32).T)       # [FF, D]
        b2_all.append(b_ff2[l].astype(f32))
        g1b1_all.append(np.stack([g1, bb1]))

    add_bq = bool(any(np.any(b != 0) for b in bq_all))

    wo_g = w_out.astype(f32) * lnf_g[None, :].astype(f32)         # [V, D]
    bo_fix = lnf_b.astype(f32) @ w_out.T.astype(f32) + b_out.astype(f32)

    wqk_np = np.stack([_pmajor(wqk_all[l]) for l in range(L)]).astype(bft)
    bk_np = np.stack([bk_all[l].reshape(DT, 128) for l in range(L)]
                     ).astype(f32)
    wp_np = np.stack([_pmajor(wp_all[l].T) for l in range(L)]).astype(bft)
    bpv_np = np.stack(bpv_all).astype(f32)
    w1_np = np.stack([_pmajor(w1_all[l]) for l in range(L)]).astype(bft)
    b1_np = np.stack([b1_all[l].reshape(FFT, 128) for l in range(L)]
                     ).astype(f32)
    w2_np = np.stack([_pmajor(w2_all[l]) for l in range(L)]).astype(bft)
    ids_np = np.asarray(input_ids).astype(np.int32)

    in_maps = []
    for c in range(N_CORES):
        g, r = divmod(c, CPG)
        rot = [(r + j) % CPG for j in range(CPG)]   # local slot j -> global qt
        tok_order = np.concatenate(
            [np.arange(q * 128, (q + 1) * 128) for q in rot])
        yidx = np.empty((128, NSTREAM * QT), np.int32)
        for s_ in range(NSTREAM):
            for j in range(QT):
                yidx[:, s_ * QT + j] = (rot[j] * NSTREAM * 128
                                        + s_ * 128 + np.arange(128))
        vsel = slice(r * VPC, (r + 1) * VPC)
        m = {
            "ids": ids_np[2 * g:2 * g + 2][:, tok_order],
            "yidx": yidx,
            "tok": tok_bf,
            "pos": _pmajor(pos_full[tok_order]),
            "wqk": wqk_np,
            "bk": bk_np,
            "wp": wp_np,
            "bpv": bpv_np,
            "w1": w1_np,
            "b1": b1_np,
            "w2": w2_np,
            "ah": alpha_h,
            "wo": _pmajor(wo_g[vsel].T).astype(bft),
            "tick": np.zeros((1, 4), f32),
        }
        if add_bq:
            m["bq"] = np.stack(bq_all).astype(f32)
        if add_b2:
            m["b2q"] = np.stack(b2_all).astype(f32)
        if not g1_triv:
            m["g1b1"] = np.stack(g1b1_all).astype(f32)
        in_maps.append(m)
    return in_maps, (add_b2, not g1_triv, add_bq), bo_fix


def kernel(**inputs) -> np.ndarray:
    in_maps, key, bo_fix = _prep_inputs(**inputs)
    runner = _get_runner(key)
    args = runner.put_inputs(in_maps)
    res = runner.run(args)
    out = np.empty((B, S, V), dtype=np.float32)
    for c in range(N_CORES):
        g, r = divmod(c, CPG)
        rot = [(r + j) % CPG for j in range(CPG)]
        tok_order = np.concatenate(
            [np.arange(q * 128, (q + 1) * 128) for q in rot])
        out[2 * g:2 * g + 2][:, tok_order, r * VPC:(r + 1) * VPC] = \
            res[c]["out"]
    if np.any(bo_fix != 0):
        out += bo_fix[None, None, :]
    return out


# make bass importable lazily for _build_program's module-level reference
import concourse.bass as bass  # noqa: E402
